# revision 13
# baseline (speedup 1.0000x reference)
"""GQA attention kernel for 8 TRN2 NeuronCores.

Problem: B=2, T=2048, D=2048, H=16 q-heads, KV=4 kv-heads, HD=128, RoPE,
non-causal softmax, out projection. f32 reference.

Sharding: 8 cores = 2 batches x 4 kv-groups. Core c handles batch c//4 and
kv-group c%4 (4 q heads + 1 kv head). Each core computes a partial output
x @ wq_g -> attention -> (heads g) @ wo_g^T: full [T, D] partial summed on
host over the 4 groups of each batch (tensor-parallel unshard).

On-device layout: everything transposed ([hd, t], hd=128=partition dim).
 - host feeds xT, wqT, wkT, wvT (d-on-partition chunks) so projections are
   plain lhsT.T @ rhs matmuls with K=d contraction, fp32r (full PE rate).
 - scores computed transposed: ST[s, t] = k^T q per s-chunk; softmax over s
   (partitions) uses exp on ACT + bf16 chunk-adds on DVE + a ones-matmul
   partition-reduce-broadcast on PE; normalization folded into the OT evac.
 - PV: OT[hd, t] += v_nat[s, hd]^T expST[s, t] per s-chunk (bf16).
 - out projection: out[t, d] = sum_h OTn_h[j, t]^T wogT[j, d] (bf16).
"""
import os
import sys

for _p in ("/opt/trn_rl_repo", "/root/.axon_site/_ro/trn_rl_repo"):
    if os.path.isdir(_p) and _p not in sys.path:
        sys.path.append(_p)

import numpy as np
import ml_dtypes

import concourse.bass as bass
import concourse.tile as tile
from concourse import bacc, mybir
from concourse.bass_utils import run_bass_kernel_spmd

B, T, D = 2, 2048, 2048
H, KV, HD = 16, 4, 128
NR = H // KV  # 4 q heads per kv group
NCORES = 8
ROPE_BASE = 10000.0
SCALE = float(HD) ** -0.5

F32R = mybir.dt.float32r
F32 = mybir.dt.float32
BF16 = mybir.dt.bfloat16

_cache = {}


def _build_nc():
    nc = bacc.Bacc("TRN2", target_bir_lowering=False, debug=False,
                   num_devices=NCORES)

    xt_e = nc.dram_tensor("xt", [128, 16, T], F32R, kind="ExternalInput").ap()
    wqt_e = nc.dram_tensor("wqt", [128, 16, NR * HD], F32R, kind="ExternalInput").ap()
    wkt_e = nc.dram_tensor("wkt", [128, 16, HD], F32R, kind="ExternalInput").ap()
    wvt_e = nc.dram_tensor("wvt", [128, 16, HD], F32R, kind="ExternalInput").ap()
    wot_e = nc.dram_tensor("wot", [128, NR, D], BF16, kind="ExternalInput").ap()
    cos_e = nc.dram_tensor("cosa", [128, T], F32R, kind="ExternalInput").ap()
    sin_e = nc.dram_tensor("sina", [128, T], F32R, kind="ExternalInput").ap()
    rotm_e = nc.dram_tensor("rotm", [128, 128], F32R, kind="ExternalInput").ap()
    ident_e = nc.dram_tensor("ident", [128, 128], BF16, kind="ExternalInput").ap()
    ones_e = nc.dram_tensor("ones", [128, 128], BF16, kind="ExternalInput").ap()
    out_e = nc.dram_tensor("out", [T, D], F32, kind="ExternalOutput").ap()

    with tile.TileContext(nc) as tc:
        import contextlib
        with contextlib.ExitStack() as ctx:
            consts = ctx.enter_context(tc.tile_pool(name="consts", bufs=1))
            weights = ctx.enter_context(tc.tile_pool(name="weights", bufs=1))
            acts = ctx.enter_context(tc.tile_pool(name="acts", bufs=1))

            cos_sb = consts.tile([128, T], F32R, tag="cos")
            sin_sb = consts.tile([128, T], F32R, tag="sin")
            rotm_sb = consts.tile([128, 128], F32R, tag="rotm")
            ident_sb = consts.tile([128, 128], BF16, tag="ident")
            ones_sb = consts.tile([128, 128], BF16, tag="ones")
            wkt_sb = weights.tile([128, 16, HD], F32R, tag="wkt")
            wvt_sb = weights.tile([128, 16, HD], F32R, tag="wvt")
            wqt_sb = weights.tile([128, 16, NR * HD], F32R, tag="wqt")
            wot_sb = weights.tile([128, NR, D], BF16, tag="wot")
            # DMA ordering: sync queue feeds the PE-critical path (wkt then
            # xt tiles); scalar queue gets everything else, smallest/most
            # urgent first, wqt split per q-head column so q-proj j starts
            # as soon as its slice lands.
            nc.sync.dma_start(out=wkt_sb, in_=wkt_e)
            nc.scalar.dma_start(out=rotm_sb, in_=rotm_e)
            nc.scalar.dma_start(out=ident_sb, in_=ident_e)
            nc.scalar.dma_start(out=ones_sb, in_=ones_e)
            nc.scalar.dma_start(out=wvt_sb, in_=wvt_e)
            for j in range(NR):
                nc.scalar.dma_start(out=wqt_sb[:, :, j * 128:(j + 1) * 128],
                                    in_=wqt_e[:, :, j * 128:(j + 1) * 128])
            nc.scalar.dma_start(out=cos_sb, in_=cos_e)
            nc.scalar.dma_start(out=sin_sb, in_=sin_e)

            qtr = [acts.tile([128, T], F32R, tag=f"qtr{j}", name=f"qtr{j}") for j in range(NR)]
            ktr = acts.tile([128, T], F32R, tag="ktr")
            v_sb = acts.tile([128, 16, HD], BF16, tag="vsb")  # v natural, s-chunked

            # ---------------- Phase 1: projections + RoPE + v transpose ----
            with tc.tile_pool(name="xt", bufs=6) as xt_pool, \
                 tc.tile_pool(name="rope", bufs=2) as rope_pool, \
                 tc.tile_pool(name="p1ps", bufs=1, space="PSUM") as p1ps, \
                 tc.tile_pool(name="rotps", bufs=2, space="PSUM") as rotps:
                for tt in range(4):
                    tsl = slice(tt * 512, (tt + 1) * 512)
                    xq = []
                    for i in range(4):
                        xti = xt_pool.tile([128, 4, 512], F32R, tag="xt")
                        nc.sync.dma_start(out=xti, in_=xt_e[:, i * 4:(i + 1) * 4, tsl])
                        xq.append(xti)
                    qps = [p1ps.tile([128, 512], F32, tag=f"qps{j}", name=f"qps{j}_{tt}") for j in range(NR)]
                    kps = p1ps.tile([128, 512], F32, tag="kps")
                    vps = p1ps.tile([128, 512], F32, tag="vps")
                    for i in range(4):
                        for dc in range(4):
                            g = i * 4 + dc
                            nc.tensor.matmul(kps, wkt_sb[:, g, :], xq[i][:, dc, :],
                                             start=(g == 0), stop=(g == 15))
                    for i in range(4):
                        for dc in range(4):
                            g = i * 4 + dc
                            nc.tensor.matmul(vps, wvt_sb[:, g, :], xq[i][:, dc, :],
                                             start=(g == 0), stop=(g == 15))
                    for j in range(NR):
                        for i in range(4):
                            for dc in range(4):
                                g = i * 4 + dc
                                nc.tensor.matmul(
                                    qps[j], wqt_sb[:, g, j * 128:(j + 1) * 128],
                                    xq[i][:, dc, :], start=(g == 0), stop=(g == 15))

                    # RoPE on k and q heads: dst = psum*cos + (rotm.T@psum)*sin
                    def rope(src, dst):
                        q_sb = rope_pool.tile([128, 512], F32R, tag="qsb",
                                              name="q_sb")
                        nc.scalar.copy(q_sb, src)
                        rot_ps = rotps.tile([128, 512], F32, tag="rot",
                                            name="rot_ps")
                        nc.tensor.matmul(rot_ps, rotm_sb, q_sb, start=True, stop=True)
                        t1 = rope_pool.tile([128, 512], F32R, tag="t1", name="t1")
                        nc.gpsimd.tensor_mul(t1, q_sb, cos_sb[:, tsl])
                        t2 = rope_pool.tile([128, 512], F32R, tag="t2", name="t2")
                        nc.vector.tensor_mul(t2, rot_ps.bitcast(F32R), sin_sb[:, tsl])
                        nc.gpsimd.tensor_add(dst, t1, t2)

                    rope(kps, ktr[:, tsl])
                    # v: copy vT psum -> sbuf bf16, PE-transpose 128-blocks
                    vt_sb = rope_pool.tile([128, 512], BF16, tag="vt")
                    nc.scalar.copy(vt_sb, vps)
                    for vb in range(4):
                        tr_ps = rotps.tile([128, 128], BF16, tag="rot")
                        nc.tensor.transpose(tr_ps, vt_sb[:, vb * 128:(vb + 1) * 128],
                                            ident_sb)
                        nc.vector.tensor_copy(v_sb[:, tt * 4 + vb, :], tr_ps)
                    for j in range(NR):
                        rope(qps[j], qtr[j][:, tsl])

            # ---------------- Phase 2+3: attention + out projection --------
            nc.sync.dma_start(out=wot_sb, in_=wot_e)
            with tc.tile_pool(name="p2sb", bufs=3) as p2sb, \
                 tc.tile_pool(name="dens", bufs=2) as dens, \
                 tc.tile_pool(name="otn", bufs=2) as otnp, \
                 tc.tile_pool(name="ostg", bufs=4) as ostg, \
                 tc.tile_pool(name="stps", bufs=2, space="PSUM") as stps, \
                 tc.tile_pool(name="otps", bufs=2, space="PSUM") as otps, \
                 tc.tile_pool(name="outps", bufs=2, space="PSUM") as outps:
                pending = [None]  # deferred softmax epilogue of previous head

                def flush_epilogue():
                    if pending[0] is not None:
                        pending[0]()
                        pending[0] = None

                for tt in range(4):
                    tsl = slice(tt * 512, (tt + 1) * 512)
                    otn_t = otnp.tile([128, NR, 512], BF16, tag="otn")
                    for h in range(NR):
                        ot_ps = otps.tile([128, 512], F32, tag="ot",
                                          name=f"ot_{tt}_{h}")
                        den_a = dens.tile([128, 2, 512], BF16, tag="dena",
                                          name=f"dena_{tt}_{h}")
                        den_b = dens.tile([128, 2, 512], BF16, tag="denb",
                                          name=f"denb_{tt}_{h}")
                        for sg in range(8):
                            st_ps = stps.tile([128, 2, 512], F32, tag="st",
                                              name=f"st_{tt}_{h}_{sg}")
                            for half in range(2):
                                sc = sg * 2 + half
                                nc.tensor.matmul(
                                    st_ps[:, half, :],
                                    ktr[:, sc * 128:(sc + 1) * 128],
                                    qtr[h][:, tsl], start=True, stop=True)
                            ex = p2sb.tile([128, 2, 512], BF16, tag="exp",
                                           name=f"ex_{tt}_{h}_{sg}")
                            nc.scalar.activation(ex, st_ps,
                                                 mybir.ActivationFunctionType.Exp,
                                                 scale=SCALE)
                            # denominator accumulation split across DVE (even
                            # superchunks) and GpSimd (odd) to halve DVE load
                            if sg == 0:
                                nc.vector.tensor_copy(den_a, ex)
                            elif sg == 1:
                                nc.gpsimd.tensor_copy(den_b, ex)
                            elif sg % 2 == 0:
                                nc.vector.tensor_add(den_a, den_a, ex)
                            else:
                                nc.gpsimd.tensor_add(den_b, den_b, ex)
                            for half in range(2):
                                sc = sg * 2 + half
                                nc.tensor.matmul(ot_ps, v_sb[:, sc, :],
                                                 ex[:, half, :],
                                                 start=(sc == 0), stop=(sc == 15))
                            if sg == 1:
                                # previous head's epilogue lands here so its
                                # exp/add chain overlaps this head's scores
                                flush_epilogue()

                        def epilogue(ot_ps=ot_ps, den_a=den_a, den_b=den_b,
                                     h=h, otn_t=otn_t, tt=tt):
                            # partition-reduce+broadcast denominator on PE,
                            # all four halves accumulated into one PSUM bank
                            bc_ps = outps.tile([128, 512], F32, tag="ops",
                                               name=f"bc_{tt}_{h}")
                            nc.tensor.matmul(bc_ps, ones_sb, den_a[:, 0, :],
                                             start=True, stop=False)
                            nc.tensor.matmul(bc_ps, ones_sb, den_a[:, 1, :],
                                             start=False, stop=False)
                            nc.tensor.matmul(bc_ps, ones_sb, den_b[:, 0, :],
                                             start=False, stop=False)
                            nc.tensor.matmul(bc_ps, ones_sb, den_b[:, 1, :],
                                             start=False, stop=True)
                            rden = dens.tile([128, 512], F32, tag="rden",
                                             name=f"rden_{tt}_{h}")
                            nc.vector.reciprocal(rden, bc_ps)
                            nc.vector.tensor_tensor(out=otn_t[:, h, :], in0=ot_ps,
                                                    in1=rden,
                                                    op=mybir.AluOpType.mult)
                        pending[0] = epilogue

                    flush_epilogue()
                    # out projection for this t-tile
                    for tkc in range(4):
                        rows = slice(tt * 512 + tkc * 128, tt * 512 + (tkc + 1) * 128)
                        for dt in range(4):
                            o_ps = outps.tile([128, 512], F32, tag="ops")
                            for hh in range(NR):
                                nc.tensor.matmul(
                                    o_ps, otn_t[:, hh, tkc * 128:(tkc + 1) * 128],
                                    wot_sb[:, hh, dt * 512:(dt + 1) * 512],
                                    start=(hh == 0), stop=(hh == NR - 1))
                            o_sb = ostg.tile([128, 512], F32, tag="ostg")
                            if dt % 2 == 0:
                                nc.vector.tensor_copy(o_sb, o_ps)
                            else:
                                nc.scalar.copy(o_sb, o_ps)
                            nc.sync.dma_start(
                                out=out_e[rows, dt * 512:(dt + 1) * 512], in_=o_sb)
    nc.compile()
    return nc


def _get_nc():
    if "nc" not in _cache:
        _cache["nc"] = _build_nc()
    return _cache["nc"]


def _host_consts():
    if "consts" in _cache:
        return _cache["consts"]
    inv = 1.0 / (ROPE_BASE ** (np.arange(0, HD, 2, dtype=np.float64) / HD))
    freqs = np.outer(np.arange(T, dtype=np.float64), inv)  # [T, 64]
    emb = np.concatenate([freqs, freqs], axis=-1)  # [T, 128]
    cos_t = np.cos(emb).T.astype(np.float32).copy()  # [128, T]
    sin_t = np.sin(emb).T.astype(np.float32).copy()
    P = np.zeros((128, 128), dtype=np.float32)
    P[:64, 64:] = -np.eye(64, dtype=np.float32)
    P[64:, :64] = np.eye(64, dtype=np.float32)
    rotm = P.T.copy()
    ident = np.eye(128, dtype=np.float32).astype(ml_dtypes.bfloat16)
    ones = np.ones((128, 128), dtype=np.float32).astype(ml_dtypes.bfloat16)
    _cache["consts"] = (cos_t, sin_t, rotm, ident, ones)
    return _cache["consts"]


def _in_maps(x, wq, wk, wv, wo):
    cos_t, sin_t, rotm, ident, ones = _host_consts()
    maps = []
    for c in range(NCORES):
        b, g = c // KV, c % KV
        xt = np.ascontiguousarray(
            x[b].reshape(T, 16, 128).transpose(2, 1, 0)).astype(np.float32)
        wq_g = wq[g * NR * HD:(g + 1) * NR * HD]  # [512, D]
        wqt = np.ascontiguousarray(wq_g.reshape(NR * HD, 16, 128).transpose(2, 1, 0))
        wk_g = wk[g * HD:(g + 1) * HD]
        wkt = np.ascontiguousarray(wk_g.reshape(HD, 16, 128).transpose(2, 1, 0))
        wv_g = wv[g * HD:(g + 1) * HD]
        wvt = np.ascontiguousarray(wv_g.reshape(HD, 16, 128).transpose(2, 1, 0))
        wo_g = wo[:, g * NR * HD:(g + 1) * NR * HD]  # [D, 512]
        wot = np.ascontiguousarray(
            wo_g.reshape(D, NR, 128).transpose(2, 1, 0)).astype(ml_dtypes.bfloat16)
        maps.append({
            "xt": xt, "wqt": wqt.astype(np.float32), "wkt": wkt.astype(np.float32),
            "wvt": wvt.astype(np.float32), "wot": wot,
            "cosa": cos_t, "sina": sin_t, "rotm": rotm,
            "ident": ident, "ones": ones,
        })
    return maps


def run_spmd(x, wq, wk, wv, wo, **kw):
    nc = _get_nc()
    maps = _in_maps(x, wq, wk, wv, wo)
    return run_bass_kernel_spmd(nc, maps, core_ids=list(range(NCORES)), **kw)


def kernel(x, wq, wk, wv, wo):
    res = run_spmd(x, wq, wk, wv, wo)
    out = np.zeros((B, T, D), dtype=np.float32)
    for c in range(NCORES):
        out[c // KV] += res.results[c]["out"]
    return out


# revision 14
# speedup vs baseline: 1.0869x; 1.0869x over previous
"""GQA attention kernel for 8 TRN2 NeuronCores.

Problem: B=2, T=2048, D=2048, H=16 q-heads, KV=4 kv-heads, HD=128, RoPE,
non-causal softmax, out projection. f32 reference.

Sharding: 8 cores = 2 batches x 4 kv-groups. Core c handles batch c//4 and
kv-group c%4 (4 q heads + 1 kv head). Each core computes a partial output
x @ wq_g -> attention -> (heads g) @ wo_g^T: full [T, D] partial summed on
host over the 4 groups of each batch (tensor-parallel unshard).

On-device layout: everything transposed ([hd, t], hd=128=partition dim).
 - host feeds xT, wqT, wkT, wvT (d-on-partition chunks) so projections are
   plain lhsT.T @ rhs matmuls with K=d contraction, fp32r (full PE rate).
 - scores computed transposed: ST[s, t] = k^T q per s-chunk; softmax over s
   (partitions) uses exp on ACT + bf16 chunk-adds on DVE + a ones-matmul
   partition-reduce-broadcast on PE; normalization folded into the OT evac.
 - PV: OT[hd, t] += v_nat[s, hd]^T expST[s, t] per s-chunk (bf16).
 - out projection: out[t, d] = sum_h OTn_h[j, t]^T wogT[j, d] (bf16).
"""
import os
import sys

for _p in ("/opt/trn_rl_repo", "/root/.axon_site/_ro/trn_rl_repo"):
    if os.path.isdir(_p) and _p not in sys.path:
        sys.path.append(_p)

import numpy as np
import ml_dtypes

import concourse.bass as bass
import concourse.tile as tile
from concourse import bacc, mybir
from concourse.bass_utils import run_bass_kernel_spmd

B, T, D = 2, 2048, 2048
H, KV, HD = 16, 4, 128
NR = H // KV  # 4 q heads per kv group
NCORES = 8
ROPE_BASE = 10000.0
SCALE = float(HD) ** -0.5

F32R = mybir.dt.float32r
F32 = mybir.dt.float32
BF16 = mybir.dt.bfloat16

_cache = {}


def _build_nc():
    nc = bacc.Bacc("TRN2", target_bir_lowering=False, debug=False,
                   num_devices=NCORES)

    xt_e = nc.dram_tensor("xt", [128, 16, T], F32R, kind="ExternalInput").ap()
    wqt_e = nc.dram_tensor("wqt", [128, 16, NR * HD], F32R, kind="ExternalInput").ap()
    wkt_e = nc.dram_tensor("wkt", [128, 16, HD], F32R, kind="ExternalInput").ap()
    wvt_e = nc.dram_tensor("wvt", [128, 16, HD], F32R, kind="ExternalInput").ap()
    wot_e = nc.dram_tensor("wot", [128, NR, D], BF16, kind="ExternalInput").ap()
    cos_e = nc.dram_tensor("cosa", [128, T], F32R, kind="ExternalInput").ap()
    sin_e = nc.dram_tensor("sina", [128, T], F32R, kind="ExternalInput").ap()
    rotm_e = nc.dram_tensor("rotm", [128, 128], F32R, kind="ExternalInput").ap()
    ident_e = nc.dram_tensor("ident", [128, 128], BF16, kind="ExternalInput").ap()
    ones_e = nc.dram_tensor("ones", [128, 128], BF16, kind="ExternalInput").ap()
    out_e = nc.dram_tensor("out", [T, D], F32, kind="ExternalOutput").ap()

    with tile.TileContext(nc) as tc:
        import contextlib
        with contextlib.ExitStack() as ctx:
            consts = ctx.enter_context(tc.tile_pool(name="consts", bufs=1))
            weights = ctx.enter_context(tc.tile_pool(name="weights", bufs=1))
            acts = ctx.enter_context(tc.tile_pool(name="acts", bufs=1))

            cos_sb = consts.tile([128, T], F32R, tag="cos")
            sin_sb = consts.tile([128, T], F32R, tag="sin")
            rotm_sb = consts.tile([128, 128], F32R, tag="rotm")
            ident_sb = consts.tile([128, 128], BF16, tag="ident")
            ones_sb = consts.tile([128, 128], BF16, tag="ones")
            wkt_sb = weights.tile([128, 16, HD], F32R, tag="wkt")
            wvt_sb = weights.tile([128, 16, HD], F32R, tag="wvt")
            wqt_sb = weights.tile([128, 16, NR * HD], F32R, tag="wqt")
            wot_sb = weights.tile([128, NR, D], BF16, tag="wot")
            # DMA ordering: sync queue feeds the PE-critical path (wkt then
            # xt tiles); scalar queue gets everything else, smallest/most
            # urgent first, wqt split per q-head column so q-proj j starts
            # as soon as its slice lands.
            nc.sync.dma_start(out=wkt_sb, in_=wkt_e)
            nc.sync.dma_start(out=wvt_sb, in_=wvt_e)
            nc.scalar.dma_start(out=rotm_sb, in_=rotm_e)
            nc.scalar.dma_start(out=ident_sb, in_=ident_e)
            nc.scalar.dma_start(out=ones_sb, in_=ones_e)
            nc.scalar.dma_start(out=cos_sb, in_=cos_e)
            nc.scalar.dma_start(out=sin_sb, in_=sin_e)
            for j in range(NR):
                nc.scalar.dma_start(out=wqt_sb[:, :, j * 128:(j + 1) * 128],
                                    in_=wqt_e[:, :, j * 128:(j + 1) * 128])

            qtr = [acts.tile([128, T], F32R, tag=f"qtr{j}", name=f"qtr{j}") for j in range(NR)]
            ktr = acts.tile([128, T], F32R, tag="ktr")
            v_sb = acts.tile([128, 16, HD], BF16, tag="vsb")  # v natural, s-chunked

            # ---------------- Phase 1: projections + RoPE + v transpose ----
            with tc.tile_pool(name="xt", bufs=6) as xt_pool, \
                 tc.tile_pool(name="rope", bufs=2) as rope_pool, \
                 tc.tile_pool(name="p1ps", bufs=1, space="PSUM") as p1ps, \
                 tc.tile_pool(name="rotps", bufs=2, space="PSUM") as rotps:
                for tt in range(4):
                    tsl = slice(tt * 512, (tt + 1) * 512)
                    xq = []
                    for i in range(4):
                        xti = xt_pool.tile([128, 4, 512], F32R, tag="xt")
                        nc.sync.dma_start(out=xti, in_=xt_e[:, i * 4:(i + 1) * 4, tsl])
                        xq.append(xti)
                    qps = [p1ps.tile([128, 512], F32, tag=f"qps{j}", name=f"qps{j}_{tt}") for j in range(NR)]
                    kps = p1ps.tile([128, 512], F32, tag="kps")
                    vps = p1ps.tile([128, 512], F32, tag="vps")
                    for i in range(4):
                        for dc in range(4):
                            g = i * 4 + dc
                            nc.tensor.matmul(kps, wkt_sb[:, g, :], xq[i][:, dc, :],
                                             start=(g == 0), stop=(g == 15))
                    for i in range(4):
                        for dc in range(4):
                            g = i * 4 + dc
                            nc.tensor.matmul(vps, wvt_sb[:, g, :], xq[i][:, dc, :],
                                             start=(g == 0), stop=(g == 15))
                    for j in range(NR):
                        for i in range(4):
                            for dc in range(4):
                                g = i * 4 + dc
                                nc.tensor.matmul(
                                    qps[j], wqt_sb[:, g, j * 128:(j + 1) * 128],
                                    xq[i][:, dc, :], start=(g == 0), stop=(g == 15))

                    # RoPE on k and q heads: dst = psum*cos + (rotm.T@psum)*sin
                    def rope(src, dst):
                        q_sb = rope_pool.tile([128, 512], F32R, tag="qsb",
                                              name="q_sb")
                        nc.scalar.copy(q_sb, src)
                        rot_ps = rotps.tile([128, 512], F32, tag="rot",
                                            name="rot_ps")
                        nc.tensor.matmul(rot_ps, rotm_sb, q_sb, start=True, stop=True)
                        t1 = rope_pool.tile([128, 512], F32R, tag="t1", name="t1")
                        nc.gpsimd.tensor_mul(t1, q_sb, cos_sb[:, tsl])
                        t2 = rope_pool.tile([128, 512], F32R, tag="t2", name="t2")
                        nc.vector.tensor_mul(t2, rot_ps.bitcast(F32R), sin_sb[:, tsl])
                        nc.vector.tensor_add(dst, t1, t2)

                    rope(kps, ktr[:, tsl])
                    # v: copy vT psum -> sbuf bf16, PE-transpose 128-blocks
                    vt_sb = rope_pool.tile([128, 512], BF16, tag="vt")
                    nc.scalar.copy(vt_sb, vps)
                    for vb in range(4):
                        tr_ps = rotps.tile([128, 128], BF16, tag="rot")
                        nc.tensor.transpose(tr_ps, vt_sb[:, vb * 128:(vb + 1) * 128],
                                            ident_sb)
                        nc.vector.tensor_copy(v_sb[:, tt * 4 + vb, :], tr_ps)
                    for j in range(NR):
                        rope(qps[j], qtr[j][:, tsl])

            # ---------------- Phase 2+3: attention + out projection --------
            nc.sync.dma_start(out=wot_sb, in_=wot_e)
            with tc.tile_pool(name="p2sb", bufs=3) as p2sb, \
                 tc.tile_pool(name="dens", bufs=2) as dens, \
                 tc.tile_pool(name="otn", bufs=2) as otnp, \
                 tc.tile_pool(name="ostg", bufs=4) as ostg, \
                 tc.tile_pool(name="stps", bufs=2, space="PSUM") as stps, \
                 tc.tile_pool(name="otps", bufs=2, space="PSUM") as otps, \
                 tc.tile_pool(name="outps", bufs=2, space="PSUM") as outps:
                pending = [None]  # deferred softmax epilogue of previous head

                def flush_epilogue():
                    if pending[0] is not None:
                        pending[0]()
                        pending[0] = None

                for tt in range(4):
                    tsl = slice(tt * 512, (tt + 1) * 512)
                    otn_t = otnp.tile([128, NR, 512], BF16, tag="otn")
                    for h in range(NR):
                        ot_ps = otps.tile([128, 512], F32, tag="ot",
                                          name=f"ot_{tt}_{h}")
                        den = dens.tile([128, 2, 512], BF16, tag="den",
                                        name=f"den_{tt}_{h}")
                        for sg in range(8):
                            st_ps = stps.tile([128, 2, 512], F32, tag="st",
                                              name=f"st_{tt}_{h}_{sg}")
                            for half in range(2):
                                sc = sg * 2 + half
                                nc.tensor.matmul(
                                    st_ps[:, half, :],
                                    ktr[:, sc * 128:(sc + 1) * 128],
                                    qtr[h][:, tsl], start=True, stop=True)
                            ex = p2sb.tile([128, 2, 512], BF16, tag="exp",
                                           name=f"ex_{tt}_{h}_{sg}")
                            nc.scalar.activation(ex, st_ps,
                                                 mybir.ActivationFunctionType.Exp,
                                                 scale=SCALE)
                            if sg == 0:
                                nc.vector.tensor_copy(den, ex)
                            else:
                                nc.vector.tensor_add(den, den, ex)
                            for half in range(2):
                                sc = sg * 2 + half
                                nc.tensor.matmul(ot_ps, v_sb[:, sc, :],
                                                 ex[:, half, :],
                                                 start=(sc == 0), stop=(sc == 15))
                            if sg == 1:
                                # previous head's epilogue lands here so its
                                # exp/add chain overlaps this head's scores
                                flush_epilogue()

                        def epilogue(ot_ps=ot_ps, den=den, h=h, otn_t=otn_t,
                                     tt=tt):
                            # partition-reduce+broadcast denominator on PE,
                            # both halves accumulated into one PSUM bank
                            bc_ps = outps.tile([128, 512], F32, tag="ops",
                                               name=f"bc_{tt}_{h}")
                            nc.tensor.matmul(bc_ps, ones_sb, den[:, 0, :],
                                             start=True, stop=False)
                            nc.tensor.matmul(bc_ps, ones_sb, den[:, 1, :],
                                             start=False, stop=True)
                            rden = dens.tile([128, 512], F32, tag="rden",
                                             name=f"rden_{tt}_{h}")
                            nc.vector.reciprocal(rden, bc_ps)
                            nc.vector.tensor_tensor(out=otn_t[:, h, :], in0=ot_ps,
                                                    in1=rden,
                                                    op=mybir.AluOpType.mult)
                        pending[0] = epilogue

                    flush_epilogue()
                    # out projection for this t-tile
                    for tkc in range(4):
                        rows = slice(tt * 512 + tkc * 128, tt * 512 + (tkc + 1) * 128)
                        for dt in range(4):
                            o_ps = outps.tile([128, 512], F32, tag="ops")
                            for hh in range(NR):
                                nc.tensor.matmul(
                                    o_ps, otn_t[:, hh, tkc * 128:(tkc + 1) * 128],
                                    wot_sb[:, hh, dt * 512:(dt + 1) * 512],
                                    start=(hh == 0), stop=(hh == NR - 1))
                            o_sb = ostg.tile([128, 512], F32, tag="ostg")
                            if dt % 2 == 0:
                                nc.vector.tensor_copy(o_sb, o_ps)
                            else:
                                nc.scalar.copy(o_sb, o_ps)
                            nc.sync.dma_start(
                                out=out_e[rows, dt * 512:(dt + 1) * 512], in_=o_sb)
    nc.compile()
    return nc


def _get_nc():
    if "nc" not in _cache:
        _cache["nc"] = _build_nc()
    return _cache["nc"]


def _host_consts():
    if "consts" in _cache:
        return _cache["consts"]
    inv = 1.0 / (ROPE_BASE ** (np.arange(0, HD, 2, dtype=np.float64) / HD))
    freqs = np.outer(np.arange(T, dtype=np.float64), inv)  # [T, 64]
    emb = np.concatenate([freqs, freqs], axis=-1)  # [T, 128]
    cos_t = np.cos(emb).T.astype(np.float32).copy()  # [128, T]
    sin_t = np.sin(emb).T.astype(np.float32).copy()
    P = np.zeros((128, 128), dtype=np.float32)
    P[:64, 64:] = -np.eye(64, dtype=np.float32)
    P[64:, :64] = np.eye(64, dtype=np.float32)
    rotm = P.T.copy()
    ident = np.eye(128, dtype=np.float32).astype(ml_dtypes.bfloat16)
    ones = np.ones((128, 128), dtype=np.float32).astype(ml_dtypes.bfloat16)
    _cache["consts"] = (cos_t, sin_t, rotm, ident, ones)
    return _cache["consts"]


def _in_maps(x, wq, wk, wv, wo):
    cos_t, sin_t, rotm, ident, ones = _host_consts()
    maps = []
    for c in range(NCORES):
        b, g = c // KV, c % KV
        xt = np.ascontiguousarray(
            x[b].reshape(T, 16, 128).transpose(2, 1, 0)).astype(np.float32)
        wq_g = wq[g * NR * HD:(g + 1) * NR * HD]  # [512, D]
        wqt = np.ascontiguousarray(wq_g.reshape(NR * HD, 16, 128).transpose(2, 1, 0))
        wk_g = wk[g * HD:(g + 1) * HD]
        wkt = np.ascontiguousarray(wk_g.reshape(HD, 16, 128).transpose(2, 1, 0))
        wv_g = wv[g * HD:(g + 1) * HD]
        wvt = np.ascontiguousarray(wv_g.reshape(HD, 16, 128).transpose(2, 1, 0))
        wo_g = wo[:, g * NR * HD:(g + 1) * NR * HD]  # [D, 512]
        wot = np.ascontiguousarray(
            wo_g.reshape(D, NR, 128).transpose(2, 1, 0)).astype(ml_dtypes.bfloat16)
        maps.append({
            "xt": xt, "wqt": wqt.astype(np.float32), "wkt": wkt.astype(np.float32),
            "wvt": wvt.astype(np.float32), "wot": wot,
            "cosa": cos_t, "sina": sin_t, "rotm": rotm,
            "ident": ident, "ones": ones,
        })
    return maps


def run_spmd(x, wq, wk, wv, wo, **kw):
    nc = _get_nc()
    maps = _in_maps(x, wq, wk, wv, wo)
    return run_bass_kernel_spmd(nc, maps, core_ids=list(range(NCORES)), **kw)


def kernel(x, wq, wk, wv, wo):
    res = run_spmd(x, wq, wk, wv, wo)
    out = np.zeros((B, T, D), dtype=np.float32)
    for c in range(NCORES):
        out[c // KV] += res.results[c]["out"]
    return out
